# revision 23
# baseline (speedup 1.0000x reference)
"""Distributed Trainium2 kernel for AttentionalPropagation (SuperGlue-style).

Reference computation (B=4, D=256, H=4, N=2048):
    q = Wq x + bq ; k = Wk s + bk ; v = Wv s + bv           (1x1 convs)
    prob = softmax(q^T k / sqrt(D))  per (b, h)
    msg  = Wm (v prob^T) + bm
    h1   = W1 [x; msg] + b1
    y    = BN(h1) * gamma + beta ; relu
    out  = W2 y + b2

Sharding: the 16 (b, h) pairs are split 2-per-core across 8 NeuronCores
(data-parallel over B x tensor-parallel over H); the only cross-core
dependency is the BatchNorm statistics (4 KB AllReduce).

Algebraic restructure (key to the op-count):
  * scores = x^T (Wq^T Wk) s, so the q- and k-projections collapse into ONE
    conv with the host-precomputed G = Wq^T Wk:  k' = G s  (+ Wq^T bk), and
    the scores GEMM streams the fp8 input x directly.
  * Since sum_m prob[m,n] = 1, the v-projection and both output convs
    commute through the softmax average:
        W1m @ (Wm (Wv (s prob) + bv) + bm) = (W1m Wm Wv) @ u + const
    with u = (s @ E) * (1/denom). The host folds W1m@Wm@Wv into the msg
    half of W1 and the constant into b1. The v-projection and the Wm conv
    disappear from the device entirely; attention computes u = softmax
    average of the RAW SOURCE rows.
  * BatchNorm folds into W2 (gamma > 0): W2 @ relu(gamma (h-mu)/sigma + b) =
    (W2 diag(gamma/sigma)) @ relu(h - mu + beta sigma/gamma).

Precision: attention runs fp8-e4m3 with DoubleRow matmuls (contracts 2
128-tiles per instruction); G is pre-scaled by 64 so its ~0.006-magnitude
entries clear e4m3's subnormal floor (undone in the exp scale). u is evacuated
at 64x (ones-lhsT value 1/64 folds the factor into the denominator) and the
msg half of W1 carries the 1/64. msg contributes only ~1% of h's variance, so
fp8 noise there is diluted ~100x; the x path (W1/W2 GEMMs) stays bf16.

Engine layout: TensorE does all GEMMs including the softmax denominators
(partition-axis sums via a ones-vector DoubleRow lhsT) and the reciprocal
partition-broadcast (K=1 matmul). ScalarE does exp (1024-wide reads across
paired PSUM banks), the BN h1-square accumulations, and half the pass-2
relus. VectorE does every PSUM evacuation plus the fast Newton reciprocal.
Pair 0's W1 GEMM is software-pipelined into pair 1's exp-bound attention
window; the BN-stats exchange is an AllGather (single ncfw phase) + local
tree sum. The framework pre/postamble (~16us) and the ~16-22us collective
latency floor are fixed costs.
"""

import os
import sys

import numpy as np

sys.path.insert(0, "/opt/trn_rl_repo")

import concourse.bass as bass
import concourse.bacc as bacc
import concourse.tile as tile
from concourse import mybir
from concourse.bass_utils import run_bass_kernel_spmd

import ml_dtypes

BF16 = ml_dtypes.bfloat16
FP8 = ml_dtypes.float8_e4m3

B, D, H, N = 4, 256, 4, 2048
EPS = 1e-5
P = 128
NCORES = 8
PAIRS_PER_CORE = (B * H) // NCORES  # 2
CT = D // P       # channel tiles for D (2)
CT2 = 2 * D // P  # channel tiles for 2D (4)
MT = N // P       # m tiles (16)
TP = MT // 2      # DoubleRow m-tile pairs (8)
NCH = 4           # n chunks of 512
CHUNK = N // NCH  # 512
WS = 64.0         # host-side scale on the fp8 attention weights

AF = mybir.ActivationFunctionType
ALU = mybir.AluOpType
PM = mybir.MatmulPerfMode
f32 = mybir.dt.float32
bf16 = mybir.dt.bfloat16
fp8 = mybir.dt.float8e4

_CACHE = {}


def build_bass() -> bass.Bass:
    nc = bacc.Bacc("TRN2", num_devices=NCORES)

    # ---- DRAM parameters (per-core shards; weights replicated) ----
    xb = nc.dram_tensor("xb", [PAIRS_PER_CORE, P, CT, N], bf16, kind="ExternalInput")
    x8 = nc.dram_tensor(
        "x8", [PAIRS_PER_CORE, P, NCH, CT, CHUNK], fp8, kind="ExternalInput"
    )
    s8 = nc.dram_tensor(
        "s8", [PAIRS_PER_CORE, P, NCH, CT, CHUNK], fp8, kind="ExternalInput"
    )
    sT8 = nc.dram_tensor(
        "sT8", [PAIRS_PER_CORE, P, MT, D], fp8, kind="ExternalInput"
    )
    gT = nc.dram_tensor("gT", [P, CT, D], fp8, kind="ExternalInput")
    w1T = nc.dram_tensor("w1T", [P, CT2, 2 * D], bf16, kind="ExternalInput")
    w2T = nc.dram_tensor("w2T", [P, CT2, D], bf16, kind="ExternalInput")
    vecs = nc.dram_tensor("vecs", [P, 24], f32, kind="ExternalInput")
    out = nc.dram_tensor("out", [PAIRS_PER_CORE, CT, P, N], bf16, kind="ExternalOutput")

    # bounce buffers for the BN-stats AllReduce + a tiny warmup AllReduce so
    # the real one (on the critical path) hits warm ncfw state.
    cc_in = nc.dram_tensor("cc_in", [P, 2 * CT2], f32)
    cc_out = nc.dram_tensor(
        "cc_out", [NCORES, P, 2 * CT2], f32, addr_space="Shared"
    )
    cw_in = nc.dram_tensor("cw_in", [1, 8], f32)
    cw_out = nc.dram_tensor("cw_out", [NCORES, 1, 8], f32, addr_space="Shared")

    with tile.TileContext(nc) as tc:
        with (
            tc.tile_pool(name="consts", bufs=1) as consts,
            tc.tile_pool(name="persist", bufs=1) as persist,
            tc.tile_pool(name="pairbuf", bufs=1) as pairbuf,
            tc.tile_pool(name="work", bufs=2) as work,
            tc.tile_pool(name="quad", bufs=2, space="PSUM") as quad,
            tc.tile_pool(name="psum", bufs=4, space="PSUM") as psum,
        ):
            # ---- weights/constants (gpsimd SWDGE overlaps the sync x/s) ----
            g_s = consts.tile([P, CT, D], fp8, tag="g_s", name="g_s")
            nc.sync.dma_start(out=g_s[:], in_=gT[:])
            w1_s = consts.tile([P, CT2, 2 * D], bf16, tag="w1_s", name="w1_s")
            nc.gpsimd.dma_start(out=w1_s[:], in_=w1T[:])
            w2_s = consts.tile([P, CT2, D], bf16, tag="w2_s", name="w2_s")
            nc.gpsimd.dma_start(out=w2_s[:], in_=w2T[:])
            vec_s = consts.tile([P, 24], f32, tag="vec_s")
            nc.gpsimd.dma_start(out=vec_s[:], in_=vecs[:])
            bkp_s = vec_s[:, 0:2]  # 64 * Wq^T bk
            b1_s = vec_s[:, 8:12]  # b1 + W1m @ (Wm bv + bm)
            b2_s = vec_s[:, 12:14]
            gm_s = vec_s[:, 14:18]
            bt_s = vec_s[:, 18:22]

            # ones lhsT for the denominator matmuls (value 1/64 folds the 64x
            # u-scale into the denominator) and a ones matrix whose rows
            # 0/32/64/96 serve as K=1 broadcast lhsT at those base partitions.
            onesd = consts.tile([P, CT, 16], fp8, tag="onesd")
            nc.vector.memset(onesd, 1.0 / WS)
            onesb = consts.tile([P, P], bf16, tag="onesb")
            nc.vector.memset(onesb, 1.0)

            # Pin the natural_log/exp ACT table set before the first Exp.
            warm = persist.tile([P, 1], f32, tag="warm")
            nc.vector.memset(warm, 1.0)
            nc.scalar.activation(warm, warm, AF.Ln)
            nc.scalar.activation(warm, warm, AF.Exp)

            pe_w = persist.tile([P, CHUNK], bf16, tag="pe_w")
            nc.vector.memset(pe_w, 0.0)
            for _ in range(10):
                pw = psum.tile([P, CHUNK], f32, tag="mm512", name="mmps")
                nc.tensor.matmul(pw, pe_w[:, 0:P], pe_w, start=True, stop=True)

            nc.gpsimd.collective_compute(
                "AllGather",
                ALU.bypass,
                replica_groups=[list(range(NCORES))],
                ins=[cw_in[:].opt()],
                outs=[cw_out[:].opt()],
            )

            # BN partials. ssq slots: one per (pair, m, j) DVE square call.
            # sigu slots: one per (pair, dh, j) u_s evacuation (accum_out).
            ssq = persist.tile([P, CT2, PAIRS_PER_CORE], f32, tag="ssq")
            sigu = persist.tile([P, CT, PAIRS_PER_CORE * NCH], f32, tag="sigu")
            sigx = persist.tile([P, CT, PAIRS_PER_CORE], bf16, tag="sigx")
            h1 = [
                persist.tile([P, CT2, N], bf16, tag=f"h1_{p}", name=f"h1_{p}")
                for p in range(PAIRS_PER_CORE)
            ]

            # ---- all input DMAs up front (both pairs) ----
            x_s, x8_s, s8_s, sT_s = [], [], [], []
            for p in range(PAIRS_PER_CORE):
                x_s.append(work.tile([P, CT, N], bf16, tag="x_s", name=f"x_s{p}"))
                x8_s.append(work.tile([P, NCH, CT, CHUNK], fp8, tag="x8_s", name=f"x8_s{p}"))
                s8_s.append(work.tile([P, NCH, CT, CHUNK], fp8, tag="s8_s", name=f"s8_s{p}"))
                sT_s.append(work.tile([P, MT, D], fp8, tag="sT_s", name=f"sT_s{p}"))
            for p in range(PAIRS_PER_CORE):
                for hh in range(2):
                    j2 = slice(hh * 2, hh * 2 + 2)
                    nc.sync.dma_start(out=s8_s[p][:, j2], in_=s8[p, :, j2])
                for hh in range(2):
                    j2 = slice(hh * 2, hh * 2 + 2)
                    nc.sync.dma_start(out=x8_s[p][:, j2], in_=x8[p, :, j2])
            for p in range(PAIRS_PER_CORE):
                for hh in range(2):
                    t8 = slice(hh * TP, hh * TP + TP)
                    nc.gpsimd.dma_start(out=sT_s[p][:, t8], in_=sT8[p, :, t8])
                    sl = slice(hh * (N // 2), (hh + 1) * (N // 2))
                    nc.gpsimd.dma_start(out=x_s[p][:, :, sl], in_=xb[p, :, :, sl])

            k8 = [None] * PAIRS_PER_CORE
            # (u_s tiles created lazily by emit_usweep)
            e_full = [None] * PAIRS_PER_CORE
            u_s = [None] * PAIRS_PER_CORE
            rb_s = [None] * PAIRS_PER_CORE
            dps = [None] * PAIRS_PER_CORE

            def emit_kconv(p):
                # k' = G s + Wq^T bk, laid out [m-tile, d-half, m%128] so
                # scores lhsT slices are contiguous per tile.
                k8[p] = pairbuf.tile([P, MT, CT, P], fp8, tag="k8", bufs=2, name=f"k8_{p}")
                for c in range(CT):
                    for j in range(NCH):
                        ps = psum.tile([P, CHUNK], f32, tag="mm512", name="mmps")
                        nc.tensor.matmul(
                            ps,
                            g_s[:, :, c * P : (c + 1) * P],
                            s8_s[p][:, j],
                            start=True,
                            stop=True,
                            perf_mode=PM.DoubleRow,
                        )
                        nc.vector.tensor_scalar_add(
                            k8[p][:, 4 * j : 4 * j + 4, c, :], ps, bkp_s[:, c : c + 1]
                        )

            def emit_denoms(p, tp_range):
                for tp in tp_range:
                    for j in range(NCH):
                        nc.tensor.matmul(
                            dps[p][j],
                            onesd[:, :, 0:1],
                            e_full[p][:, tp, j, :, :],
                            start=(tp == 0),
                            stop=(tp == TP - 1),
                            perf_mode=PM.DoubleRow,
                        )

            def emit_attention(p, inline_denoms, fill_cb=None):
                # S^T tiles (m on partitions) via weight-stationary k'-tiles;
                # exp reads paired PSUM banks (1024 wide) into e_full. The
                # softmax denominators (ones-lhsT partition sums on TensorE)
                # either accumulate inline per finished t-pair (pair 0) or
                # run post-loop (pair 1, whose PSUM budget feeds fill_cb).
                e_full[p] = pairbuf.tile(
                    [P, TP, NCH, 2, CHUNK], fp8, tag="e_full",
                    name=f"e_full{p}", bufs=1,
                )
                dps[p] = [
                    psum.tile([1, CHUNK], f32, tag="mm512", name=f"dps{p}{j}")
                    for j in range(NCH)
                ] if inline_denoms else None
                for t in range(MT):
                    for jj in range(2):
                        q2 = quad.tile([P, 2, CHUNK], f32, tag="q2", name="q2")
                        for u in range(2):
                            nc.tensor.matmul(
                                q2[:, u, :],
                                k8[p][:, t, :, :],
                                x8_s[p][:, 2 * jj + u],
                                start=True,
                                stop=True,
                                perf_mode=PM.DoubleRow,
                            )
                        nc.scalar.activation(
                            e_full[p][:, t // 2, 2 * jj : 2 * jj + 2, t % 2, :],
                            q2[:],
                            AF.Exp,
                            scale=1.0 / (16.0 * WS),
                        )
                    if inline_denoms and t % 2 == 1:
                        emit_denoms(p, [t // 2])
                    if fill_cb is not None and t % 4 == 3:
                        fill_cb(t // 4)
                if not inline_denoms:
                    dps[p] = [
                        psum.tile([1, CHUNK], f32, tag="mm512", name=f"dps{p}{j}")
                        for j in range(NCH)
                    ]
                    emit_denoms(p, range(TP))

            def emit_recips(p):
                # rec = 1/(denom/64): broadcast the denominator across
                # partitions with a K=1 matmul, then a fast DVE Newton
                # reciprocal on the full 128-partition tile. No ScalarE, no
                # ACT-table switches.
                den_s = pairbuf.tile([1, NCH, CHUNK], bf16, tag="den_s", bufs=1, name=f"den_s{p}")
                rb_s[p] = pairbuf.tile([P, NCH, CHUNK], f32, tag="rb_s", bufs=1, name=f"rb_s{p}")
                for j in range(NCH):
                    nc.vector.tensor_copy(den_s[:, j, :], dps[p][j])
                for j in range(NCH):
                    rbp = psum.tile([P, CHUNK], f32, tag="mm512", name="mmps")
                    nc.tensor.matmul(
                        rbp,
                        onesb[0:1, 0:P],
                        den_s[:, j, :],
                        start=True,
                        stop=True,
                    )
                    with nc.allow_low_precision(reason="softmax reciprocal"):
                        nc.vector.reciprocal_approx_fast(rb_s[p][:, j, :], rbp)

            def emit_usweep(p, dh, j):
                # u = (s @ E) * rec; u_s = 64u bf16. accum_out collects the
                # column sums for the BN mean (linearity trick).
                if u_s[p] is None:
                    u_s[p] = pairbuf.tile(
                        [P, CT, N], bf16, tag="u_s", name=f"u_s{p}"
                    )
                up = psum.tile([P, CHUNK], f32, tag="mm512", name="mmps")
                for tp in range(TP):
                    nc.tensor.matmul(
                        up,
                        sT_s[p][:, 2 * tp : 2 * tp + 2, dh * P : (dh + 1) * P],
                        e_full[p][:, tp, j, :, :],
                        start=(tp == 0),
                        stop=(tp == TP - 1),
                        perf_mode=PM.DoubleRow,
                    )
                slot = p * NCH + j
                nc.vector.scalar_tensor_tensor(
                    u_s[p][:, dh, j * CHUNK : (j + 1) * CHUNK],
                    up,
                    0.0,
                    rb_s[p][:, j, :],
                    op0=ALU.add,
                    op1=ALU.mult,
                    accum_out=sigu[:, dh, slot : slot + 1],
                )

            def emit_uphase(p):
                for dh in range(CT):
                    for j in range(NCH):
                        emit_usweep(p, dh, j)

            sq_scr = persist.tile([P, N], bf16, tag="sq_scr")

            def emit_squares(p):
                # BN sum-of-squares: one wide Square+accum per h1 row, on
                # ScalarE during windows where it would otherwise idle.
                for m in range(CT2):
                    nc.scalar.activation(
                        sq_scr,
                        h1[p][:, m, :],
                        AF.Square,
                        accum_out=ssq[:, m, p : p + 1],
                    )

            def emit_w1_block(p, m, use_quads=False):
                # One output-channel tile of W1eff over [x; u] (bf16): h1 to
                # SBUF (DVE). With use_quads (pair-1 tail, attention done),
                # two of the four in-flight PSUMs come from the quad pool to
                # halve evacuation-latency stalls on the start matmuls.
                w1_rhs = [
                    x_s[p][:, 0, :], x_s[p][:, 1, :],
                    u_s[p][:, 0, :], u_s[p][:, 1, :],
                ]
                if use_quads:
                    qt = quad.tile([P, 2, CHUNK], f32, tag="q2", name="q2")
                    pss = [
                        qt[:, 0, :], qt[:, 1, :],
                        psum.tile([P, CHUNK], f32, tag="mm512", name="mmps"),
                        psum.tile([P, CHUNK], f32, tag="mm512", name="mmps"),
                    ]
                else:
                    pss = [
                        psum.tile([P, CHUNK], f32, tag="mm512", name="mmps")
                        for _ in range(NCH)
                    ]
                for k in range(CT2):
                    lhsT = w1_s[:, k, m * P : (m + 1) * P]
                    for j in range(NCH):
                        nc.tensor.matmul(
                            pss[j],
                            lhsT,
                            w1_rhs[k][:, j * CHUNK : (j + 1) * CHUNK],
                            start=(k == 0),
                            stop=(k == CT2 - 1),
                        )
                for j in range(NCH):
                    sl = slice(j * CHUNK, (j + 1) * CHUNK)
                    nc.vector.tensor_scalar_add(
                        h1[p][:, m, sl], pss[j], b1_s[:, m : m + 1]
                    )

            def emit_sigx(p):
                with nc.allow_low_precision(reason="bf16 colsums feed bf16 GEMM"):
                    for c in range(CT):
                        nc.vector.reduce_sum(
                            sigx[:, c, p : p + 1],
                            x_s[p][:, c, :],
                            axis=mybir.AxisListType.X,
                        )

            # ---- software-pipelined schedule: pair 1's exp-bound attention
            # window absorbs pair 0's W1 GEMM.
            emit_kconv(0)
            emit_attention(0, inline_denoms=True)
            emit_recips(0)
            emit_kconv(1)
            emit_uphase(0)
            emit_sigx(0)
            emit_attention(1, inline_denoms=False,
                           fill_cb=lambda m: emit_w1_block(0, m))
            emit_recips(1)
            emit_squares(0)
            emit_uphase(1)
            emit_sigx(1)
            for m in range(CT2):
                emit_w1_block(1, m, use_quads=True)
                nc.scalar.activation(
                    sq_scr,
                    h1[1][:, m, :],
                    AF.Square,
                    accum_out=ssq[:, m, 1:2],
                )
            # Preload the Ln ACT table while ScalarE idles so the BN-stats Ln
            # after the AllReduce pays no table switch.
            nc.scalar.activation(warm, warm, AF.Ln)

            # ---- BN statistics: sum_n h1 = W1eff @ colsum([x; u]) + N*b1 ----
            sig_t = persist.tile([P, CT2], bf16, tag="sig_t")
            nc.vector.tensor_add(sig_t[:, 0:CT], sigx[:, :, 0], sigx[:, :, 1])
            with nc.allow_low_precision(reason="bf16 colsums feed bf16 GEMM"):
                for c in range(CT):
                    nc.vector.reduce_sum(
                        sig_t[:, CT + c : CT + c + 1],
                        sigu[:, c, :],
                        axis=mybir.AxisListType.X,
                    )
            pstat = psum.tile([P, CHUNK], f32, tag="mm512", name="pstat")
            for m in range(CT2):
                for k in range(CT2):
                    nc.tensor.matmul(
                        pstat[:, m : m + 1],
                        w1_s[:, k, m * P : (m + 1) * P],
                        sig_t[:, k : k + 1],
                        start=(k == 0),
                        stop=(k == CT2 - 1),
                    )
            stats_l = persist.tile([P, 2 * CT2], f32, tag="stats_l")
            nb1 = persist.tile([P, CT2], f32, tag="nb1")
            nc.vector.tensor_scalar_mul(nb1, b1_s, float(2 * N))
            rg = persist.tile([P, CT2], f32, tag="rg")
            with nc.allow_low_precision(reason="gamma reciprocal, f32"):
                nc.vector.reciprocal(rg, gm_s)
            nc.vector.tensor_add(stats_l[:, 0:CT2], pstat[:, 0:CT2], nb1)
            for m in range(CT2):
                nc.vector.tensor_add(
                    stats_l[:, CT2 + m : CT2 + m + 1],
                    ssq[:, m, 0:1],
                    ssq[:, m, 1:2],
                )
            # Cross-core exchange of the 4 KB BN stats: AllGather (single
            # ncfw phase, no reduce step) + a local 8-way tree sum.
            nc.sync.dma_start(out=cc_in[:], in_=stats_l[:])
            nc.gpsimd.collective_compute(
                "AllGather",
                ALU.bypass,
                replica_groups=[list(range(NCORES))],
                ins=[cc_in[:].opt()],
                outs=[cc_out[:].opt()],
            )
            stats_a = persist.tile([P, NCORES, 2 * CT2], f32, tag="stats_a")
            for c in range(NCORES):
                eng = nc.sync if c % 2 == 0 else nc.gpsimd
                eng.dma_start(out=stats_a[:, c, :], in_=cc_out[c])
            for step in (4, 2, 1):
                nc.vector.tensor_add(
                    stats_a[:, 0:step, :],
                    stats_a[:, 0:step, :],
                    stats_a[:, step : 2 * step, :],
                )
            stats_g = stats_a[:, 0, :]

            count = float(B * H * N)
            mom = persist.tile([P, 2 * CT2], f32, tag="mom")
            nc.vector.tensor_scalar_mul(mom, stats_g, 1.0 / count)
            var = persist.tile([P, CT2], f32, tag="var")
            nc.vector.tensor_mul(var, mom[:, 0:CT2], mom[:, 0:CT2])
            nc.vector.tensor_sub(var, mom[:, CT2 : 2 * CT2], var)
            nc.vector.tensor_scalar_add(var, var, EPS)
            # rsqrt = exp(-0.5 ln(var+eps)): Ln table preloaded above.
            lnv = persist.tile([P, CT2], f32, tag="lnv")
            nc.scalar.activation(lnv, var, AF.Ln)
            inv = persist.tile([P, CT2], f32, tag="inv")
            nc.scalar.activation(inv, lnv, AF.Exp, scale=-0.5)
            # sigma = (var+eps)*rsqrt(var+eps): no second Exp needed.
            sg = persist.tile([P, CT2], f32, tag="sg")
            nc.vector.tensor_mul(sg, var, inv)
            # BN folded into W2 (gamma > 0): w2f = w2 * (gamma/sigma) per
            # input channel; relu threshold thr = beta*sigma/gamma - mu.
            scl = persist.tile([P, CT2], f32, tag="scl")
            nc.vector.tensor_mul(scl, gm_s, inv)
            thr = persist.tile([P, CT2], f32, tag="thr")
            nc.vector.tensor_mul(thr, bt_s, sg)
            nc.vector.tensor_mul(thr, thr, rg)
            nc.vector.tensor_sub(thr, thr, mom[:, 0:CT2])
            w2f = persist.tile([P, CT2, D], bf16, tag="w2f")
            for k in range(CT2):
                nc.vector.tensor_scalar_mul(
                    w2f[:, k, :], w2_s[:, k, :], scl[:, k : k + 1]
                )

            # ---- pass 2: relu(h1 + thr) per n-chunk (ScalarE/DVE split),
            # double-buffered chunk tiles so the pairs overlap, then the
            # rescaled W2 GEMM with PSUM slots from both pools.
            for p in range(PAIRS_PER_CORE):
                o_big = work.tile(
                    [P, CT, N], bf16, tag="o_big", name=f"o_big{p}"
                )
                for j in range(NCH):
                    sl = slice(j * CHUNK, (j + 1) * CHUNK)
                    h1n = work.tile(
                        [P, CT2, CHUNK], bf16, tag="h1n", name=f"h1n{p}{j}"
                    )
                    for m in range(CT2):
                        if m < 2:
                            nc.scalar.activation(
                                h1n[:, m, :],
                                h1[p][:, m, sl],
                                AF.Relu,
                                bias=thr[:, m : m + 1],
                            )
                        else:
                            nc.vector.tensor_scalar(
                                h1n[:, m, :],
                                h1[p][:, m, sl],
                                thr[:, m : m + 1],
                                0.0,
                                op0=ALU.add,
                                op1=ALU.max,
                            )
                    for c in range(CT):
                        if c == 0:
                            qt = quad.tile([P, 2, CHUNK], f32, tag="q2", name="q2")
                            ps = qt[:, 0, :]
                        else:
                            ps = psum.tile([P, CHUNK], f32, tag="mm512", name="mmps")
                        for k in range(CT2):
                            nc.tensor.matmul(
                                ps,
                                w2f[:, k, c * P : (c + 1) * P],
                                h1n[:, k, :],
                                start=(k == 0),
                                stop=(k == CT2 - 1),
                            )
                        nc.vector.tensor_scalar_add(
                            o_big[:, c, sl], ps, b2_s[:, c : c + 1]
                        )
                        nc.sync.dma_start(out=out[p, c, :, sl], in_=o_big[:, c, sl])

    nc.finalize()
    return nc
def _get_nc():
    if "nc" not in _CACHE:
        _CACHE["nc"] = build_bass()
    return _CACHE["nc"]


def _prep_inputs(inputs):
    """Host-side shard/fold/transpose/cast. Returns in_maps for the 8 cores."""
    x = np.asarray(inputs["x"], np.float32)
    source = np.asarray(inputs["source"], np.float32)

    # [B, D, H, N] -> [B*H pairs, P, CT, N] (partition-major)
    def to_pairs(a):
        a = a.transpose(0, 2, 1, 3).reshape(B * H, CT, P, N)
        return np.ascontiguousarray(a.transpose(0, 2, 1, 3))

    xp_f = to_pairs(x)
    sp_f = to_pairs(source)
    xp = xp_f.astype(BF16)

    # [pairs, P, CT, N] -> [pairs, P, NCH, CT, CHUNK] (DoubleRow-contiguous)
    def to_chunks(a):
        return np.ascontiguousarray(
            a.reshape(B * H, P, CT, NCH, CHUNK).transpose(0, 1, 3, 2, 4)
        ).astype(FP8)

    xp8 = to_chunks(xp_f)
    sp8 = to_chunks(sp_f)
    # s^T: [pairs, P(m%128), MT, D]
    sT = source.transpose(0, 2, 3, 1).reshape(B * H, MT, P, D)
    sT8 = np.ascontiguousarray(sT.transpose(0, 2, 1, 3)).astype(FP8)

    def lhsT(w, dt, scale=1.0):
        wT = np.ascontiguousarray(np.asarray(w, np.float32).T * scale)
        cin, cout = wT.shape
        a = wT.reshape(cin // P, P, cout).transpose(1, 0, 2)
        return np.ascontiguousarray(a).astype(dt)

    def vcol(b):
        return np.asarray(b, np.float32).reshape(-1, P).T  # [P, kt]

    Wq = np.asarray(inputs["Wq"], np.float32)
    Wk = np.asarray(inputs["Wk"], np.float32)
    Wv = np.asarray(inputs["Wv"], np.float32)
    Wm = np.asarray(inputs["Wm"], np.float32)
    W1 = np.asarray(inputs["W1"], np.float32)
    G = Wq.T @ Wk
    WU = W1[:, D:] @ Wm @ Wv
    W1eff = np.concatenate([W1[:, :D], WU / WS], axis=1)
    bm_eff = Wm @ np.asarray(inputs["bv"], np.float32) + np.asarray(
        inputs["bm"], np.float32
    )
    b1_eff = np.asarray(inputs["b1"], np.float32) + W1[:, D:] @ bm_eff
    bkp = WS * (Wq.T @ np.asarray(inputs["bk"], np.float32))

    vecs = np.zeros((P, 24), np.float32)
    vecs[:, 0:2] = vcol(bkp)
    vecs[:, 8:12] = vcol(b1_eff)
    vecs[:, 12:14] = vcol(inputs["b2"])
    vecs[:, 14:18] = vcol(inputs["gamma"])
    vecs[:, 18:22] = vcol(inputs["beta"])

    common = {
        "gT": lhsT(G, FP8, WS),
        "w1T": lhsT(W1eff, BF16),
        "w2T": lhsT(inputs["W2"], BF16),
        "vecs": vecs,
    }
    in_maps = []
    for i in range(NCORES):
        m = dict(common)
        pp = slice(i * PAIRS_PER_CORE, (i + 1) * PAIRS_PER_CORE)
        m["xb"] = np.ascontiguousarray(xp[pp])
        m["x8"] = np.ascontiguousarray(xp8[pp])
        m["s8"] = np.ascontiguousarray(sp8[pp])
        m["sT8"] = np.ascontiguousarray(sT8[pp])
        in_maps.append(m)
    return in_maps


def run_on_hw(inputs, trace=False, **kw):
    nc = _get_nc()
    in_maps = _prep_inputs(inputs)
    res = run_bass_kernel_spmd(
        nc, in_maps, core_ids=list(range(NCORES)), trace=trace, **kw
    )
    outs = res.results
    full = np.empty((B, H, D, N), np.float32)
    for i in range(NCORES):
        o = np.asarray(outs[i]["out"]).astype(np.float32).reshape(PAIRS_PER_CORE, D, N)
        for jp in range(PAIRS_PER_CORE):
            gp = i * PAIRS_PER_CORE + jp
            full[gp // H, gp % H] = o[jp]
    return full.transpose(0, 2, 1, 3), res


def kernel(**inputs) -> np.ndarray:
    out, _ = run_on_hw(inputs, trace=False)
    return out


# revision 24
# speedup vs baseline: 1.0042x; 1.0042x over previous
"""Distributed Trainium2 kernel for AttentionalPropagation (SuperGlue-style).

Reference computation (B=4, D=256, H=4, N=2048):
    q = Wq x + bq ; k = Wk s + bk ; v = Wv s + bv           (1x1 convs)
    prob = softmax(q^T k / sqrt(D))  per (b, h)
    msg  = Wm (v prob^T) + bm
    h1   = W1 [x; msg] + b1
    y    = BN(h1) * gamma + beta ; relu
    out  = W2 y + b2

Sharding: the 16 (b, h) pairs are split 2-per-core across 8 NeuronCores
(data-parallel over B x tensor-parallel over H); the only cross-core
dependency is the BatchNorm statistics (4 KB AllReduce).

Algebraic restructure (key to the op-count):
  * scores = x^T (Wq^T Wk) s, so the q- and k-projections collapse into ONE
    conv with the host-precomputed G = Wq^T Wk:  k' = G s  (+ Wq^T bk), and
    the scores GEMM streams the fp8 input x directly.
  * Since sum_m prob[m,n] = 1, the v-projection and both output convs
    commute through the softmax average:
        W1m @ (Wm (Wv (s prob) + bv) + bm) = (W1m Wm Wv) @ u + const
    with u = (s @ E) * (1/denom). The host folds W1m@Wm@Wv into the msg
    half of W1 and the constant into b1. The v-projection and the Wm conv
    disappear from the device entirely; attention computes u = softmax
    average of the RAW SOURCE rows.
  * BatchNorm folds into W2 (gamma > 0): W2 @ relu(gamma (h-mu)/sigma + b) =
    (W2 diag(gamma/sigma)) @ relu(h - mu + beta sigma/gamma).

Precision: attention runs fp8-e4m3 with DoubleRow matmuls (contracts 2
128-tiles per instruction); G is pre-scaled by 64 so its ~0.006-magnitude
entries clear e4m3's subnormal floor (undone in the exp scale). u is evacuated
at 64x (ones-lhsT value 1/64 folds the factor into the denominator) and the
msg half of W1 carries the 1/64. msg contributes only ~1% of h's variance, so
fp8 noise there is diluted ~100x; the x path (W1/W2 GEMMs) stays bf16.

Engine layout: TensorE does all GEMMs including the softmax denominators
(partition-axis sums via a ones-vector DoubleRow lhsT) and the reciprocal
partition-broadcast (K=1 matmul). ScalarE does exp (1024-wide reads across
paired PSUM banks), the BN h1-square accumulations, and half the pass-2
relus. VectorE does every PSUM evacuation plus the fast Newton reciprocal.
Pair 0's W1 GEMM is software-pipelined into pair 1's exp-bound attention
window; the BN-stats exchange is an AllGather (single ncfw phase) + local
tree sum. The framework pre/postamble (~16us) and the ~16-22us collective
latency floor are fixed costs.
"""

import os
import sys

import numpy as np

sys.path.insert(0, "/opt/trn_rl_repo")

import concourse.bass as bass
import concourse.bacc as bacc
import concourse.tile as tile
from concourse import mybir
from concourse.bass_utils import run_bass_kernel_spmd

import ml_dtypes

BF16 = ml_dtypes.bfloat16
FP8 = ml_dtypes.float8_e4m3

B, D, H, N = 4, 256, 4, 2048
EPS = 1e-5
P = 128
NCORES = 8
PAIRS_PER_CORE = (B * H) // NCORES  # 2
CT = D // P       # channel tiles for D (2)
CT2 = 2 * D // P  # channel tiles for 2D (4)
MT = N // P       # m tiles (16)
TP = MT // 2      # DoubleRow m-tile pairs (8)
NCH = 4           # n chunks of 512
CHUNK = N // NCH  # 512
WS = 64.0         # host-side scale on the fp8 attention weights

AF = mybir.ActivationFunctionType
ALU = mybir.AluOpType
PM = mybir.MatmulPerfMode
f32 = mybir.dt.float32
bf16 = mybir.dt.bfloat16
fp8 = mybir.dt.float8e4

_CACHE = {}


def build_bass() -> bass.Bass:
    nc = bacc.Bacc("TRN2", num_devices=NCORES)

    # ---- DRAM parameters (per-core shards; weights replicated) ----
    xb = nc.dram_tensor("xb", [PAIRS_PER_CORE, P, CT, N], bf16, kind="ExternalInput")
    x8 = nc.dram_tensor(
        "x8", [PAIRS_PER_CORE, P, NCH, CT, CHUNK], fp8, kind="ExternalInput"
    )
    s8 = nc.dram_tensor(
        "s8", [PAIRS_PER_CORE, P, NCH, CT, CHUNK], fp8, kind="ExternalInput"
    )
    sT8 = nc.dram_tensor(
        "sT8", [PAIRS_PER_CORE, P, MT, D], fp8, kind="ExternalInput"
    )
    gT = nc.dram_tensor("gT", [P, CT, D], fp8, kind="ExternalInput")
    w1T = nc.dram_tensor("w1T", [P, CT2, 2 * D], bf16, kind="ExternalInput")
    w2T = nc.dram_tensor("w2T", [P, CT2, D], bf16, kind="ExternalInput")
    vecs = nc.dram_tensor("vecs", [P, 24], f32, kind="ExternalInput")
    out = nc.dram_tensor("out", [PAIRS_PER_CORE, CT, P, N], bf16, kind="ExternalOutput")

    # bounce buffers for the BN-stats AllReduce + a tiny warmup AllReduce so
    # the real one (on the critical path) hits warm ncfw state.
    cc_in = nc.dram_tensor("cc_in", [P, 2 * CT2], f32)
    cc_out = nc.dram_tensor(
        "cc_out", [NCORES, P, 2 * CT2], f32, addr_space="Shared"
    )
    cw_in = nc.dram_tensor("cw_in", [1, 8], f32)
    cw_out = nc.dram_tensor("cw_out", [NCORES, 1, 8], f32, addr_space="Shared")

    with tile.TileContext(nc) as tc:
        with (
            tc.tile_pool(name="consts", bufs=1) as consts,
            tc.tile_pool(name="persist", bufs=1) as persist,
            tc.tile_pool(name="pairbuf", bufs=1) as pairbuf,
            tc.tile_pool(name="work", bufs=2) as work,
            tc.tile_pool(name="quad", bufs=2, space="PSUM") as quad,
            tc.tile_pool(name="psum", bufs=4, space="PSUM") as psum,
        ):
            # ---- weights/constants (gpsimd SWDGE overlaps the sync x/s) ----
            g_s = consts.tile([P, CT, D], fp8, tag="g_s", name="g_s")
            nc.sync.dma_start(out=g_s[:], in_=gT[:])
            w1_s = consts.tile([P, CT2, 2 * D], bf16, tag="w1_s", name="w1_s")
            nc.gpsimd.dma_start(out=w1_s[:], in_=w1T[:])
            w2_s = consts.tile([P, CT2, D], bf16, tag="w2_s", name="w2_s")
            nc.gpsimd.dma_start(out=w2_s[:], in_=w2T[:])
            vec_s = consts.tile([P, 24], f32, tag="vec_s")
            nc.gpsimd.dma_start(out=vec_s[:], in_=vecs[:])
            bkp_s = vec_s[:, 0:2]  # 64 * Wq^T bk
            b1_s = vec_s[:, 8:12]  # b1 + W1m @ (Wm bv + bm)
            b2_s = vec_s[:, 12:14]
            gm_s = vec_s[:, 14:18]
            bt_s = vec_s[:, 18:22]

            # ones lhsT for the denominator matmuls (value 1/64 folds the 64x
            # u-scale into the denominator) and a ones matrix whose rows
            # 0/32/64/96 serve as K=1 broadcast lhsT at those base partitions.
            onesd = consts.tile([P, CT, 16], fp8, tag="onesd")
            nc.vector.memset(onesd, 1.0 / WS)
            onesb = consts.tile([P, P], bf16, tag="onesb")
            nc.vector.memset(onesb, 1.0)

            # Pin the natural_log/exp ACT table set before the first Exp.
            warm = persist.tile([P, 1], f32, tag="warm")
            nc.vector.memset(warm, 1.0)
            nc.scalar.activation(warm, warm, AF.Ln)
            nc.scalar.activation(warm, warm, AF.Exp)

            pe_w = persist.tile([P, CHUNK], bf16, tag="pe_w")
            nc.vector.memset(pe_w, 0.0)
            for _ in range(6):
                pw = psum.tile([P, CHUNK], f32, tag="mm512", name="mmps")
                nc.tensor.matmul(pw, pe_w[:, 0:P], pe_w, start=True, stop=True)

            nc.gpsimd.collective_compute(
                "AllGather",
                ALU.bypass,
                replica_groups=[list(range(NCORES))],
                ins=[cw_in[:].opt()],
                outs=[cw_out[:].opt()],
            )

            # BN partials. ssq slots: one per (pair, m, j) DVE square call.
            # sigu slots: one per (pair, dh, j) u_s evacuation (accum_out).
            ssq = persist.tile([P, CT2, PAIRS_PER_CORE], f32, tag="ssq")
            sigu = persist.tile([P, CT, PAIRS_PER_CORE * NCH], f32, tag="sigu")
            sigx = persist.tile([P, CT, PAIRS_PER_CORE], bf16, tag="sigx")
            h1 = [
                persist.tile([P, CT2, N], bf16, tag=f"h1_{p}", name=f"h1_{p}")
                for p in range(PAIRS_PER_CORE)
            ]

            # ---- all input DMAs up front (both pairs) ----
            x_s, x8_s, s8_s, sT_s = [], [], [], []
            for p in range(PAIRS_PER_CORE):
                x_s.append(work.tile([P, CT, N], bf16, tag="x_s", name=f"x_s{p}"))
                x8_s.append(work.tile([P, NCH, CT, CHUNK], fp8, tag="x8_s", name=f"x8_s{p}"))
                s8_s.append(work.tile([P, NCH, CT, CHUNK], fp8, tag="s8_s", name=f"s8_s{p}"))
                sT_s.append(work.tile([P, MT, D], fp8, tag="sT_s", name=f"sT_s{p}"))
            for p in range(PAIRS_PER_CORE):
                for hh in range(2):
                    j2 = slice(hh * 2, hh * 2 + 2)
                    nc.sync.dma_start(out=s8_s[p][:, j2], in_=s8[p, :, j2])
                for hh in range(2):
                    j2 = slice(hh * 2, hh * 2 + 2)
                    nc.sync.dma_start(out=x8_s[p][:, j2], in_=x8[p, :, j2])
            for p in range(PAIRS_PER_CORE):
                for hh in range(2):
                    t8 = slice(hh * TP, hh * TP + TP)
                    nc.gpsimd.dma_start(out=sT_s[p][:, t8], in_=sT8[p, :, t8])
                    sl = slice(hh * (N // 2), (hh + 1) * (N // 2))
                    nc.gpsimd.dma_start(out=x_s[p][:, :, sl], in_=xb[p, :, :, sl])

            k8 = [None] * PAIRS_PER_CORE
            # (u_s tiles created lazily by emit_usweep)
            e_full = [None] * PAIRS_PER_CORE
            u_s = [None] * PAIRS_PER_CORE
            rb_s = [None] * PAIRS_PER_CORE
            dps = [None] * PAIRS_PER_CORE

            def emit_kconv(p):
                # k' = G s + Wq^T bk, laid out [m-tile, d-half, m%128] so
                # scores lhsT slices are contiguous per tile.
                k8[p] = pairbuf.tile([P, MT, CT, P], fp8, tag="k8", bufs=2, name=f"k8_{p}")
                for c in range(CT):
                    for j in range(NCH):
                        ps = psum.tile([P, CHUNK], f32, tag="mm512", name="mmps")
                        nc.tensor.matmul(
                            ps,
                            g_s[:, :, c * P : (c + 1) * P],
                            s8_s[p][:, j],
                            start=True,
                            stop=True,
                            perf_mode=PM.DoubleRow,
                        )
                        nc.vector.tensor_scalar_add(
                            k8[p][:, 4 * j : 4 * j + 4, c, :], ps, bkp_s[:, c : c + 1]
                        )

            def emit_denoms(p, tp_range):
                for tp in tp_range:
                    for j in range(NCH):
                        nc.tensor.matmul(
                            dps[p][j],
                            onesd[:, :, 0:1],
                            e_full[p][:, tp, j, :, :],
                            start=(tp == 0),
                            stop=(tp == TP - 1),
                            perf_mode=PM.DoubleRow,
                        )

            def emit_attention(p, inline_denoms, fill_cb=None):
                # S^T tiles (m on partitions) via weight-stationary k'-tiles;
                # exp reads paired PSUM banks (1024 wide) into e_full. The
                # softmax denominators (ones-lhsT partition sums on TensorE)
                # either accumulate inline per finished t-pair (pair 0) or
                # run post-loop (pair 1, whose PSUM budget feeds fill_cb).
                e_full[p] = pairbuf.tile(
                    [P, TP, NCH, 2, CHUNK], fp8, tag="e_full",
                    name=f"e_full{p}", bufs=1,
                )
                dps[p] = [
                    psum.tile([1, CHUNK], f32, tag="mm512", name=f"dps{p}{j}")
                    for j in range(NCH)
                ] if inline_denoms else None
                for t in range(MT):
                    for jj in range(2):
                        q2 = quad.tile([P, 2, CHUNK], f32, tag="q2", name="q2")
                        for u in range(2):
                            nc.tensor.matmul(
                                q2[:, u, :],
                                k8[p][:, t, :, :],
                                x8_s[p][:, 2 * jj + u],
                                start=True,
                                stop=True,
                                perf_mode=PM.DoubleRow,
                            )
                        nc.scalar.activation(
                            e_full[p][:, t // 2, 2 * jj : 2 * jj + 2, t % 2, :],
                            q2[:],
                            AF.Exp,
                            scale=1.0 / (16.0 * WS),
                        )
                    if inline_denoms and t % 2 == 1:
                        emit_denoms(p, [t // 2])
                    if fill_cb is not None and t % 4 == 2:
                        fill_cb(t // 4)
                if not inline_denoms:
                    dps[p] = [
                        psum.tile([1, CHUNK], f32, tag="mm512", name=f"dps{p}{j}")
                        for j in range(NCH)
                    ]
                    emit_denoms(p, range(TP))

            def emit_recips(p):
                # rec = 1/(denom/64): broadcast the denominator across
                # partitions with a K=1 matmul, then a fast DVE Newton
                # reciprocal on the full 128-partition tile. No ScalarE, no
                # ACT-table switches.
                den_s = pairbuf.tile([1, NCH, CHUNK], bf16, tag="den_s", bufs=1, name=f"den_s{p}")
                rb_s[p] = pairbuf.tile([P, NCH, CHUNK], f32, tag="rb_s", bufs=1, name=f"rb_s{p}")
                for j in range(NCH):
                    nc.vector.tensor_copy(den_s[:, j, :], dps[p][j])
                for j in range(NCH):
                    rbp = psum.tile([P, CHUNK], f32, tag="mm512", name="mmps")
                    nc.tensor.matmul(
                        rbp,
                        onesb[0:1, 0:P],
                        den_s[:, j, :],
                        start=True,
                        stop=True,
                    )
                    with nc.allow_low_precision(reason="softmax reciprocal"):
                        nc.vector.reciprocal_approx_fast(rb_s[p][:, j, :], rbp)

            def emit_usweep(p, dh, j):
                # u = (s @ E) * rec; u_s = 64u bf16. accum_out collects the
                # column sums for the BN mean (linearity trick).
                if u_s[p] is None:
                    u_s[p] = pairbuf.tile(
                        [P, CT, N], bf16, tag="u_s", name=f"u_s{p}"
                    )
                up = psum.tile([P, CHUNK], f32, tag="mm512", name="mmps")
                for tp in range(TP):
                    nc.tensor.matmul(
                        up,
                        sT_s[p][:, 2 * tp : 2 * tp + 2, dh * P : (dh + 1) * P],
                        e_full[p][:, tp, j, :, :],
                        start=(tp == 0),
                        stop=(tp == TP - 1),
                        perf_mode=PM.DoubleRow,
                    )
                slot = p * NCH + j
                nc.vector.scalar_tensor_tensor(
                    u_s[p][:, dh, j * CHUNK : (j + 1) * CHUNK],
                    up,
                    0.0,
                    rb_s[p][:, j, :],
                    op0=ALU.add,
                    op1=ALU.mult,
                    accum_out=sigu[:, dh, slot : slot + 1],
                )

            def emit_uphase(p):
                for dh in range(CT):
                    for j in range(NCH):
                        emit_usweep(p, dh, j)

            sq_scr = persist.tile([P, N], bf16, tag="sq_scr")

            def emit_squares(p):
                # BN sum-of-squares: one wide Square+accum per h1 row, on
                # ScalarE during windows where it would otherwise idle.
                for m in range(CT2):
                    nc.scalar.activation(
                        sq_scr,
                        h1[p][:, m, :],
                        AF.Square,
                        accum_out=ssq[:, m, p : p + 1],
                    )

            def emit_w1_block(p, m, use_quads=False):
                # One output-channel tile of W1eff over [x; u] (bf16): h1 to
                # SBUF (DVE). With use_quads (pair-1 tail, attention done),
                # two of the four in-flight PSUMs come from the quad pool to
                # halve evacuation-latency stalls on the start matmuls.
                w1_rhs = [
                    x_s[p][:, 0, :], x_s[p][:, 1, :],
                    u_s[p][:, 0, :], u_s[p][:, 1, :],
                ]
                if use_quads:
                    qt = quad.tile([P, 2, CHUNK], f32, tag="q2", name="q2")
                    pss = [
                        qt[:, 0, :], qt[:, 1, :],
                        psum.tile([P, CHUNK], f32, tag="mm512", name="mmps"),
                        psum.tile([P, CHUNK], f32, tag="mm512", name="mmps"),
                    ]
                else:
                    pss = [
                        psum.tile([P, CHUNK], f32, tag="mm512", name="mmps")
                        for _ in range(NCH)
                    ]
                for k in range(CT2):
                    lhsT = w1_s[:, k, m * P : (m + 1) * P]
                    for j in range(NCH):
                        nc.tensor.matmul(
                            pss[j],
                            lhsT,
                            w1_rhs[k][:, j * CHUNK : (j + 1) * CHUNK],
                            start=(k == 0),
                            stop=(k == CT2 - 1),
                        )
                for j in range(NCH):
                    sl = slice(j * CHUNK, (j + 1) * CHUNK)
                    nc.vector.tensor_scalar_add(
                        h1[p][:, m, sl], pss[j], b1_s[:, m : m + 1]
                    )

            def emit_sigx(p):
                with nc.allow_low_precision(reason="bf16 colsums feed bf16 GEMM"):
                    for c in range(CT):
                        nc.vector.reduce_sum(
                            sigx[:, c, p : p + 1],
                            x_s[p][:, c, :],
                            axis=mybir.AxisListType.X,
                        )

            # ---- software-pipelined schedule: pair 1's exp-bound attention
            # window absorbs pair 0's W1 GEMM.
            emit_kconv(0)
            emit_attention(0, inline_denoms=True)
            emit_recips(0)
            emit_kconv(1)
            emit_uphase(0)
            emit_sigx(0)
            emit_attention(1, inline_denoms=False,
                           fill_cb=lambda m: emit_w1_block(0, m))
            emit_recips(1)
            emit_squares(0)
            emit_uphase(1)
            emit_sigx(1)
            for m in range(CT2):
                emit_w1_block(1, m, use_quads=True)
                nc.scalar.activation(
                    sq_scr,
                    h1[1][:, m, :],
                    AF.Square,
                    accum_out=ssq[:, m, 1:2],
                )
            # Preload the Ln ACT table while ScalarE idles so the BN-stats Ln
            # after the AllReduce pays no table switch.
            nc.scalar.activation(warm, warm, AF.Ln)

            # ---- BN statistics: sum_n h1 = W1eff @ colsum([x; u]) + N*b1 ----
            sig_t = persist.tile([P, CT2], bf16, tag="sig_t")
            nc.vector.tensor_add(sig_t[:, 0:CT], sigx[:, :, 0], sigx[:, :, 1])
            with nc.allow_low_precision(reason="bf16 colsums feed bf16 GEMM"):
                for c in range(CT):
                    nc.vector.reduce_sum(
                        sig_t[:, CT + c : CT + c + 1],
                        sigu[:, c, :],
                        axis=mybir.AxisListType.X,
                    )
            pstat = psum.tile([P, CHUNK], f32, tag="mm512", name="pstat")
            for m in range(CT2):
                for k in range(CT2):
                    nc.tensor.matmul(
                        pstat[:, m : m + 1],
                        w1_s[:, k, m * P : (m + 1) * P],
                        sig_t[:, k : k + 1],
                        start=(k == 0),
                        stop=(k == CT2 - 1),
                    )
            stats_l = persist.tile([P, 2 * CT2], f32, tag="stats_l")
            nb1 = persist.tile([P, CT2], f32, tag="nb1")
            nc.vector.tensor_scalar_mul(nb1, b1_s, float(2 * N))
            rg = persist.tile([P, CT2], f32, tag="rg")
            with nc.allow_low_precision(reason="gamma reciprocal, f32"):
                nc.vector.reciprocal(rg, gm_s)
            nc.vector.tensor_add(stats_l[:, 0:CT2], pstat[:, 0:CT2], nb1)
            for m in range(CT2):
                nc.vector.tensor_add(
                    stats_l[:, CT2 + m : CT2 + m + 1],
                    ssq[:, m, 0:1],
                    ssq[:, m, 1:2],
                )
            # Cross-core exchange of the 4 KB BN stats: AllGather (single
            # ncfw phase, no reduce step) + a local 8-way tree sum.
            nc.sync.dma_start(out=cc_in[:], in_=stats_l[:])
            nc.gpsimd.collective_compute(
                "AllGather",
                ALU.bypass,
                replica_groups=[list(range(NCORES))],
                ins=[cc_in[:].opt()],
                outs=[cc_out[:].opt()],
            )
            stats_a = persist.tile([P, NCORES, 2 * CT2], f32, tag="stats_a")
            for c in range(NCORES):
                eng = nc.sync if c % 2 == 0 else nc.gpsimd
                eng.dma_start(out=stats_a[:, c, :], in_=cc_out[c])
            for step in (4, 2, 1):
                nc.vector.tensor_add(
                    stats_a[:, 0:step, :],
                    stats_a[:, 0:step, :],
                    stats_a[:, step : 2 * step, :],
                )
            stats_g = stats_a[:, 0, :]

            count = float(B * H * N)
            mom = persist.tile([P, 2 * CT2], f32, tag="mom")
            nc.vector.tensor_scalar_mul(mom, stats_g, 1.0 / count)
            var = persist.tile([P, CT2], f32, tag="var")
            nc.vector.tensor_mul(var, mom[:, 0:CT2], mom[:, 0:CT2])
            nc.vector.tensor_sub(var, mom[:, CT2 : 2 * CT2], var)
            nc.vector.tensor_scalar_add(var, var, EPS)
            # rsqrt = exp(-0.5 ln(var+eps)): Ln table preloaded above.
            lnv = persist.tile([P, CT2], f32, tag="lnv")
            nc.scalar.activation(lnv, var, AF.Ln)
            inv = persist.tile([P, CT2], f32, tag="inv")
            nc.scalar.activation(inv, lnv, AF.Exp, scale=-0.5)
            # sigma = (var+eps)*rsqrt(var+eps): no second Exp needed.
            sg = persist.tile([P, CT2], f32, tag="sg")
            nc.vector.tensor_mul(sg, var, inv)
            # BN folded into W2 (gamma > 0): w2f = w2 * (gamma/sigma) per
            # input channel; relu threshold thr = beta*sigma/gamma - mu.
            scl = persist.tile([P, CT2], f32, tag="scl")
            nc.vector.tensor_mul(scl, gm_s, inv)
            thr = persist.tile([P, CT2], f32, tag="thr")
            nc.vector.tensor_mul(thr, bt_s, sg)
            nc.vector.tensor_mul(thr, thr, rg)
            nc.vector.tensor_sub(thr, thr, mom[:, 0:CT2])
            w2f = persist.tile([P, CT2, D], bf16, tag="w2f")
            for k in range(CT2):
                nc.vector.tensor_scalar_mul(
                    w2f[:, k, :], w2_s[:, k, :], scl[:, k : k + 1]
                )

            # ---- pass 2: relu(h1 + thr) per n-chunk (ScalarE/DVE split),
            # double-buffered chunk tiles so the pairs overlap, then the
            # rescaled W2 GEMM with PSUM slots from both pools.
            for p in range(PAIRS_PER_CORE):
                o_big = work.tile(
                    [P, CT, N], bf16, tag="o_big", name=f"o_big{p}"
                )
                for j in range(NCH):
                    sl = slice(j * CHUNK, (j + 1) * CHUNK)
                    h1n = work.tile(
                        [P, CT2, CHUNK], bf16, tag="h1n", name=f"h1n{p}{j}"
                    )
                    for m in range(CT2):
                        if m < 2:
                            nc.scalar.activation(
                                h1n[:, m, :],
                                h1[p][:, m, sl],
                                AF.Relu,
                                bias=thr[:, m : m + 1],
                            )
                        else:
                            nc.vector.tensor_scalar(
                                h1n[:, m, :],
                                h1[p][:, m, sl],
                                thr[:, m : m + 1],
                                0.0,
                                op0=ALU.add,
                                op1=ALU.max,
                            )
                    for c in range(CT):
                        if c == 0:
                            qt = quad.tile([P, 2, CHUNK], f32, tag="q2", name="q2")
                            ps = qt[:, 0, :]
                        else:
                            ps = psum.tile([P, CHUNK], f32, tag="mm512", name="mmps")
                        for k in range(CT2):
                            nc.tensor.matmul(
                                ps,
                                w2f[:, k, c * P : (c + 1) * P],
                                h1n[:, k, :],
                                start=(k == 0),
                                stop=(k == CT2 - 1),
                            )
                        nc.vector.tensor_scalar_add(
                            o_big[:, c, sl], ps, b2_s[:, c : c + 1]
                        )
                        nc.sync.dma_start(out=out[p, c, :, sl], in_=o_big[:, c, sl])

    nc.finalize()
    return nc
def _get_nc():
    if "nc" not in _CACHE:
        _CACHE["nc"] = build_bass()
    return _CACHE["nc"]


def _prep_inputs(inputs):
    """Host-side shard/fold/transpose/cast. Returns in_maps for the 8 cores."""
    x = np.asarray(inputs["x"], np.float32)
    source = np.asarray(inputs["source"], np.float32)

    # [B, D, H, N] -> [B*H pairs, P, CT, N] (partition-major)
    def to_pairs(a):
        a = a.transpose(0, 2, 1, 3).reshape(B * H, CT, P, N)
        return np.ascontiguousarray(a.transpose(0, 2, 1, 3))

    xp_f = to_pairs(x)
    sp_f = to_pairs(source)
    xp = xp_f.astype(BF16)

    # [pairs, P, CT, N] -> [pairs, P, NCH, CT, CHUNK] (DoubleRow-contiguous)
    def to_chunks(a):
        return np.ascontiguousarray(
            a.reshape(B * H, P, CT, NCH, CHUNK).transpose(0, 1, 3, 2, 4)
        ).astype(FP8)

    xp8 = to_chunks(xp_f)
    sp8 = to_chunks(sp_f)
    # s^T: [pairs, P(m%128), MT, D]
    sT = source.transpose(0, 2, 3, 1).reshape(B * H, MT, P, D)
    sT8 = np.ascontiguousarray(sT.transpose(0, 2, 1, 3)).astype(FP8)

    def lhsT(w, dt, scale=1.0):
        wT = np.ascontiguousarray(np.asarray(w, np.float32).T * scale)
        cin, cout = wT.shape
        a = wT.reshape(cin // P, P, cout).transpose(1, 0, 2)
        return np.ascontiguousarray(a).astype(dt)

    def vcol(b):
        return np.asarray(b, np.float32).reshape(-1, P).T  # [P, kt]

    Wq = np.asarray(inputs["Wq"], np.float32)
    Wk = np.asarray(inputs["Wk"], np.float32)
    Wv = np.asarray(inputs["Wv"], np.float32)
    Wm = np.asarray(inputs["Wm"], np.float32)
    W1 = np.asarray(inputs["W1"], np.float32)
    G = Wq.T @ Wk
    WU = W1[:, D:] @ Wm @ Wv
    W1eff = np.concatenate([W1[:, :D], WU / WS], axis=1)
    bm_eff = Wm @ np.asarray(inputs["bv"], np.float32) + np.asarray(
        inputs["bm"], np.float32
    )
    b1_eff = np.asarray(inputs["b1"], np.float32) + W1[:, D:] @ bm_eff
    bkp = WS * (Wq.T @ np.asarray(inputs["bk"], np.float32))

    vecs = np.zeros((P, 24), np.float32)
    vecs[:, 0:2] = vcol(bkp)
    vecs[:, 8:12] = vcol(b1_eff)
    vecs[:, 12:14] = vcol(inputs["b2"])
    vecs[:, 14:18] = vcol(inputs["gamma"])
    vecs[:, 18:22] = vcol(inputs["beta"])

    common = {
        "gT": lhsT(G, FP8, WS),
        "w1T": lhsT(W1eff, BF16),
        "w2T": lhsT(inputs["W2"], BF16),
        "vecs": vecs,
    }
    in_maps = []
    for i in range(NCORES):
        m = dict(common)
        pp = slice(i * PAIRS_PER_CORE, (i + 1) * PAIRS_PER_CORE)
        m["xb"] = np.ascontiguousarray(xp[pp])
        m["x8"] = np.ascontiguousarray(xp8[pp])
        m["s8"] = np.ascontiguousarray(sp8[pp])
        m["sT8"] = np.ascontiguousarray(sT8[pp])
        in_maps.append(m)
    return in_maps


def run_on_hw(inputs, trace=False, **kw):
    nc = _get_nc()
    in_maps = _prep_inputs(inputs)
    res = run_bass_kernel_spmd(
        nc, in_maps, core_ids=list(range(NCORES)), trace=trace, **kw
    )
    outs = res.results
    full = np.empty((B, H, D, N), np.float32)
    for i in range(NCORES):
        o = np.asarray(outs[i]["out"]).astype(np.float32).reshape(PAIRS_PER_CORE, D, N)
        for jp in range(PAIRS_PER_CORE):
            gp = i * PAIRS_PER_CORE + jp
            full[gp // H, gp % H] = o[jp]
    return full.transpose(0, 2, 1, 3), res


def kernel(**inputs) -> np.ndarray:
    out, _ = run_on_hw(inputs, trace=False)
    return out


# revision 25
# speedup vs baseline: 1.0914x; 1.0869x over previous
"""Distributed Trainium2 kernel for AttentionalPropagation (SuperGlue-style).

Reference computation (B=4, D=256, H=4, N=2048):
    q = Wq x + bq ; k = Wk s + bk ; v = Wv s + bv           (1x1 convs)
    prob = softmax(q^T k / sqrt(D))  per (b, h)
    msg  = Wm (v prob^T) + bm
    h1   = W1 [x; msg] + b1
    y    = BN(h1) * gamma + beta ; relu
    out  = W2 y + b2

Sharding: the 16 (b, h) pairs are split 2-per-core across 8 NeuronCores
(data-parallel over B x tensor-parallel over H); the only cross-core
dependency is the BatchNorm statistics (4 KB AllReduce).

Algebraic restructure (key to the op-count):
  * scores = x^T (Wq^T Wk) s, so the q- and k-projections collapse into ONE
    conv with the host-precomputed G = Wq^T Wk:  k' = G s  (+ Wq^T bk), and
    the scores GEMM streams the fp8 input x directly.
  * Since sum_m prob[m,n] = 1, the v-projection and both output convs
    commute through the softmax average:
        W1m @ (Wm (Wv (s prob) + bv) + bm) = (W1m Wm Wv) @ u + const
    with u = (s @ E) * (1/denom). The host folds W1m@Wm@Wv into the msg
    half of W1 and the constant into b1. The v-projection and the Wm conv
    disappear from the device entirely; attention computes u = softmax
    average of the RAW SOURCE rows.
  * BatchNorm folds into W2 (gamma > 0): W2 @ relu(gamma (h-mu)/sigma + b) =
    (W2 diag(gamma/sigma)) @ relu(h - mu + beta sigma/gamma).

Precision: attention runs fp8-e4m3 with DoubleRow matmuls (contracts 2
128-tiles per instruction); G is pre-scaled by 64 so its ~0.006-magnitude
entries clear e4m3's subnormal floor (undone in the exp scale). u is evacuated
at 64x (ones-lhsT value 1/64 folds the factor into the denominator) and the
msg half of W1 carries the 1/64. msg contributes only ~1% of h's variance, so
fp8 noise there is diluted ~100x; the x path (W1/W2 GEMMs) stays bf16.

Engine layout: TensorE does all GEMMs including the softmax denominators
(partition-axis sums via a ones-vector DoubleRow lhsT) and the reciprocal
partition-broadcast (K=1 matmul). ScalarE does exp (1024-wide reads across
paired PSUM banks), the BN h1-square accumulations, and half the pass-2
relus. VectorE does every PSUM evacuation plus the fast Newton reciprocal.
Pair 0's W1 GEMM is software-pipelined into pair 1's exp-bound attention
window; the BN-stats exchange is an AllGather (single ncfw phase) + local
tree sum. The framework pre/postamble (~16us) and the ~16-22us collective
latency floor are fixed costs.
"""

import os
import sys

import numpy as np

sys.path.insert(0, "/opt/trn_rl_repo")

import concourse.bass as bass
import concourse.bacc as bacc
import concourse.tile as tile
from concourse import mybir
from concourse.bass_utils import run_bass_kernel_spmd

import ml_dtypes

BF16 = ml_dtypes.bfloat16
FP8 = ml_dtypes.float8_e4m3

B, D, H, N = 4, 256, 4, 2048
EPS = 1e-5
P = 128
NCORES = 8
PAIRS_PER_CORE = (B * H) // NCORES  # 2
CT = D // P       # channel tiles for D (2)
CT2 = 2 * D // P  # channel tiles for 2D (4)
MT = N // P       # m tiles (16)
TP = MT // 2      # DoubleRow m-tile pairs (8)
NCH = 4           # n chunks of 512
CHUNK = N // NCH  # 512
WS = 64.0         # host-side scale on the fp8 attention weights

AF = mybir.ActivationFunctionType
ALU = mybir.AluOpType
PM = mybir.MatmulPerfMode
f32 = mybir.dt.float32
bf16 = mybir.dt.bfloat16
fp8 = mybir.dt.float8e4

_CACHE = {}


def build_bass() -> bass.Bass:
    nc = bacc.Bacc("TRN2", num_devices=NCORES)

    # ---- DRAM parameters (per-core shards; weights replicated) ----
    xb = nc.dram_tensor("xb", [PAIRS_PER_CORE, P, CT, N], bf16, kind="ExternalInput")
    x8 = nc.dram_tensor(
        "x8", [PAIRS_PER_CORE, P, NCH, CT, CHUNK], fp8, kind="ExternalInput"
    )
    s8 = nc.dram_tensor(
        "s8", [PAIRS_PER_CORE, P, NCH, CT, CHUNK], fp8, kind="ExternalInput"
    )
    sT8 = nc.dram_tensor(
        "sT8", [PAIRS_PER_CORE, P, MT, D], fp8, kind="ExternalInput"
    )
    gT = nc.dram_tensor("gT", [P, CT, D], fp8, kind="ExternalInput")
    w1T = nc.dram_tensor("w1T", [P, CT2, 2 * D], bf16, kind="ExternalInput")
    w2T = nc.dram_tensor("w2T", [P, CT2, D], bf16, kind="ExternalInput")
    vecs = nc.dram_tensor("vecs", [P, 24], f32, kind="ExternalInput")
    out = nc.dram_tensor("out", [PAIRS_PER_CORE, CT, P, N], bf16, kind="ExternalOutput")

    # bounce buffers for the BN-stats AllReduce + a tiny warmup AllReduce so
    # the real one (on the critical path) hits warm ncfw state.
    cc_in = nc.dram_tensor("cc_in", [P, 2 * CT2], f32)
    cc_out = nc.dram_tensor(
        "cc_out", [NCORES, P, 2 * CT2], f32, addr_space="Shared"
    )
    cw_in = nc.dram_tensor("cw_in", [1, 8], f32)
    cw_out = nc.dram_tensor("cw_out", [NCORES, 1, 8], f32, addr_space="Shared")

    with tile.TileContext(nc) as tc:
        with (
            tc.tile_pool(name="consts", bufs=1) as consts,
            tc.tile_pool(name="persist", bufs=1) as persist,
            tc.tile_pool(name="pairbuf", bufs=1) as pairbuf,
            tc.tile_pool(name="work", bufs=2) as work,
            tc.tile_pool(name="quad", bufs=2, space="PSUM") as quad,
            tc.tile_pool(name="psum", bufs=4, space="PSUM") as psum,
        ):
            # ---- weights/constants (gpsimd SWDGE overlaps the sync x/s) ----
            g_s = consts.tile([P, CT, D], fp8, tag="g_s", name="g_s")
            nc.sync.dma_start(out=g_s[:], in_=gT[:])
            w1_s = consts.tile([P, CT2, 2 * D], bf16, tag="w1_s", name="w1_s")
            nc.gpsimd.dma_start(out=w1_s[:], in_=w1T[:])
            w2_s = consts.tile([P, CT2, D], bf16, tag="w2_s", name="w2_s")
            nc.gpsimd.dma_start(out=w2_s[:], in_=w2T[:])
            vec_s = consts.tile([P, 24], f32, tag="vec_s")
            nc.gpsimd.dma_start(out=vec_s[:], in_=vecs[:])
            bkp_s = vec_s[:, 0:2]  # 64 * Wq^T bk
            b1_s = vec_s[:, 8:12]  # b1 + W1m @ (Wm bv + bm)
            b2_s = vec_s[:, 12:14]
            gm_s = vec_s[:, 14:18]
            bt_s = vec_s[:, 18:22]

            # ones lhsT for the denominator matmuls (value 1/64 folds the 64x
            # u-scale into the denominator) and a ones matrix whose rows
            # 0/32/64/96 serve as K=1 broadcast lhsT at those base partitions.
            onesd = consts.tile([P, CT, 16], fp8, tag="onesd")
            nc.vector.memset(onesd, 1.0 / WS)
            onesb = consts.tile([P, P], bf16, tag="onesb")
            nc.vector.memset(onesb, 1.0)

            # Pin the natural_log/exp ACT table set before the first Exp.
            warm = persist.tile([P, 1], f32, tag="warm")
            nc.vector.memset(warm, 1.0)
            nc.scalar.activation(warm, warm, AF.Ln)
            nc.scalar.activation(warm, warm, AF.Exp)

            pe_w = persist.tile([P, CHUNK], bf16, tag="pe_w")
            nc.vector.memset(pe_w, 0.0)
            for _ in range(10):
                pw = psum.tile([P, CHUNK], f32, tag="mm512", name="mmps")
                nc.tensor.matmul(pw, pe_w[:, 0:P], pe_w, start=True, stop=True)

            nc.gpsimd.collective_compute(
                "AllGather",
                ALU.bypass,
                replica_groups=[list(range(NCORES))],
                ins=[cw_in[:].opt()],
                outs=[cw_out[:].opt()],
            )

            # BN partials. ssq slots: one per (pair, m, j) DVE square call.
            # sigu slots: one per (pair, dh, j) u_s evacuation (accum_out).
            ssq = persist.tile([P, CT2, PAIRS_PER_CORE], f32, tag="ssq")
            sigu = persist.tile([P, CT, PAIRS_PER_CORE * NCH], f32, tag="sigu")
            sigx = persist.tile([P, CT, PAIRS_PER_CORE], bf16, tag="sigx")
            h1 = [
                persist.tile([P, CT2, N], bf16, tag=f"h1_{p}", name=f"h1_{p}")
                for p in range(PAIRS_PER_CORE)
            ]

            # ---- all input DMAs up front (both pairs) ----
            x_s, x8_s, s8_s, sT_s = [], [], [], []
            for p in range(PAIRS_PER_CORE):
                x_s.append(work.tile([P, CT, N], bf16, tag="x_s", name=f"x_s{p}"))
                x8_s.append(work.tile([P, NCH, CT, CHUNK], fp8, tag="x8_s", name=f"x8_s{p}"))
                s8_s.append(work.tile([P, NCH, CT, CHUNK], fp8, tag="s8_s", name=f"s8_s{p}"))
                sT_s.append(work.tile([P, MT, D], fp8, tag="sT_s", name=f"sT_s{p}"))
            for p in range(PAIRS_PER_CORE):
                for hh in range(2):
                    j2 = slice(hh * 2, hh * 2 + 2)
                    nc.sync.dma_start(out=s8_s[p][:, j2], in_=s8[p, :, j2])
                for hh in range(2):
                    j2 = slice(hh * 2, hh * 2 + 2)
                    nc.sync.dma_start(out=x8_s[p][:, j2], in_=x8[p, :, j2])
            for p in range(PAIRS_PER_CORE):
                for hh in range(2):
                    t8 = slice(hh * TP, hh * TP + TP)
                    nc.gpsimd.dma_start(out=sT_s[p][:, t8], in_=sT8[p, :, t8])
                    sl = slice(hh * (N // 2), (hh + 1) * (N // 2))
                    nc.gpsimd.dma_start(out=x_s[p][:, :, sl], in_=xb[p, :, :, sl])

            k8 = [None] * PAIRS_PER_CORE
            # (u_s tiles created lazily by emit_usweep)
            e_full = [None] * PAIRS_PER_CORE
            u_s = [None] * PAIRS_PER_CORE
            rb_s = [None] * PAIRS_PER_CORE
            dps = [None] * PAIRS_PER_CORE

            def emit_kconv(p):
                # k' = G s + Wq^T bk, laid out [m-tile, d-half, m%128] so
                # scores lhsT slices are contiguous per tile.
                k8[p] = pairbuf.tile([P, MT, CT, P], fp8, tag="k8", bufs=2, name=f"k8_{p}")
                for c in range(CT):
                    for j in range(NCH):
                        ps = psum.tile([P, CHUNK], f32, tag="mm512", name="mmps")
                        nc.tensor.matmul(
                            ps,
                            g_s[:, :, c * P : (c + 1) * P],
                            s8_s[p][:, j],
                            start=True,
                            stop=True,
                            perf_mode=PM.DoubleRow,
                        )
                        nc.vector.tensor_scalar_add(
                            k8[p][:, 4 * j : 4 * j + 4, c, :], ps, bkp_s[:, c : c + 1]
                        )

            def emit_denoms(p, tp_range):
                for tp in tp_range:
                    for j in range(NCH):
                        nc.tensor.matmul(
                            dps[p][j],
                            onesd[:, :, 0:1],
                            e_full[p][:, tp, j, :, :],
                            start=(tp == 0),
                            stop=(tp == TP - 1),
                            perf_mode=PM.DoubleRow,
                        )

            def emit_attention(p, inline_denoms, fill_cb=None):
                # S^T tiles (m on partitions) via weight-stationary k'-tiles;
                # exp reads paired PSUM banks (1024 wide) into e_full. The
                # softmax denominators (ones-lhsT partition sums on TensorE)
                # either accumulate inline per finished t-pair (pair 0) or
                # run post-loop (pair 1, whose PSUM budget feeds fill_cb).
                e_full[p] = pairbuf.tile(
                    [P, TP, NCH, 2, CHUNK], fp8, tag="e_full",
                    name=f"e_full{p}", bufs=1,
                )
                dps[p] = [
                    psum.tile([1, CHUNK], f32, tag="mm512", name=f"dps{p}{j}")
                    for j in range(NCH)
                ] if inline_denoms else None
                for t in range(MT):
                    for jj in range(2):
                        q2 = quad.tile([P, 2, CHUNK], f32, tag="q2", name="q2")
                        for u in range(2):
                            nc.tensor.matmul(
                                q2[:, u, :],
                                k8[p][:, t, :, :],
                                x8_s[p][:, 2 * jj + u],
                                start=True,
                                stop=True,
                                perf_mode=PM.DoubleRow,
                            )
                        nc.scalar.activation(
                            e_full[p][:, t // 2, 2 * jj : 2 * jj + 2, t % 2, :],
                            q2[:],
                            AF.Exp,
                            scale=1.0 / (16.0 * WS),
                        )
                    if inline_denoms and t % 2 == 1:
                        emit_denoms(p, [t // 2])
                    if fill_cb is not None and t % 4 == 3:
                        fill_cb(t // 4)
                if not inline_denoms:
                    dps[p] = [
                        psum.tile([1, CHUNK], f32, tag="mm512", name=f"dps{p}{j}")
                        for j in range(NCH)
                    ]
                    emit_denoms(p, range(TP))

            def emit_recips(p):
                # rec = 1/(denom/64): broadcast the denominator across
                # partitions with a K=1 matmul, then a fast DVE Newton
                # reciprocal on the full 128-partition tile. No ScalarE, no
                # ACT-table switches.
                den_s = pairbuf.tile([1, NCH, CHUNK], bf16, tag="den_s", bufs=1, name=f"den_s{p}")
                rb_s[p] = pairbuf.tile([P, NCH, CHUNK], f32, tag="rb_s", bufs=1, name=f"rb_s{p}")
                for j in range(NCH):
                    nc.vector.tensor_copy(den_s[:, j, :], dps[p][j])
                for j in range(NCH):
                    rbp = psum.tile([P, CHUNK], f32, tag="mm512", name="mmps")
                    nc.tensor.matmul(
                        rbp,
                        onesb[0:1, 0:P],
                        den_s[:, j, :],
                        start=True,
                        stop=True,
                    )
                    with nc.allow_low_precision(reason="softmax reciprocal"):
                        nc.vector.reciprocal_approx_fast(rb_s[p][:, j, :], rbp)

            def emit_usweep(p, dh, j):
                # u = (s @ E) * rec; u_s = 64u bf16. accum_out collects the
                # column sums for the BN mean (linearity trick).
                if u_s[p] is None:
                    u_s[p] = pairbuf.tile(
                        [P, CT, N], bf16, tag="u_s", name=f"u_s{p}"
                    )
                up = psum.tile([P, CHUNK], f32, tag="mm512", name="mmps")
                for tp in range(TP):
                    nc.tensor.matmul(
                        up,
                        sT_s[p][:, 2 * tp : 2 * tp + 2, dh * P : (dh + 1) * P],
                        e_full[p][:, tp, j, :, :],
                        start=(tp == 0),
                        stop=(tp == TP - 1),
                        perf_mode=PM.DoubleRow,
                    )
                slot = p * NCH + j
                nc.vector.scalar_tensor_tensor(
                    u_s[p][:, dh, j * CHUNK : (j + 1) * CHUNK],
                    up,
                    0.0,
                    rb_s[p][:, j, :],
                    op0=ALU.add,
                    op1=ALU.mult,
                    accum_out=sigu[:, dh, slot : slot + 1],
                )

            def emit_uphase(p):
                for dh in range(CT):
                    for j in range(NCH):
                        emit_usweep(p, dh, j)

            sq_scr = persist.tile([P, N], bf16, tag="sq_scr")

            def emit_squares(p):
                # BN sum-of-squares: one wide Square+accum per h1 row, on
                # ScalarE during windows where it would otherwise idle.
                for m in range(CT2):
                    nc.scalar.activation(
                        sq_scr,
                        h1[p][:, m, :],
                        AF.Square,
                        accum_out=ssq[:, m, p : p + 1],
                    )

            def emit_w1_block(p, m, use_quads=False):
                # One output-channel tile of W1eff over [x; u] (bf16): h1 to
                # SBUF (DVE). With use_quads (pair-1 tail, attention done),
                # two of the four in-flight PSUMs come from the quad pool to
                # halve evacuation-latency stalls on the start matmuls.
                w1_rhs = [
                    x_s[p][:, 0, :], x_s[p][:, 1, :],
                    u_s[p][:, 0, :], u_s[p][:, 1, :],
                ]
                if use_quads:
                    qt = quad.tile([P, 2, CHUNK], f32, tag="q2", name="q2")
                    pss = [
                        qt[:, 0, :], qt[:, 1, :],
                        psum.tile([P, CHUNK], f32, tag="mm512", name="mmps"),
                        psum.tile([P, CHUNK], f32, tag="mm512", name="mmps"),
                    ]
                else:
                    pss = [
                        psum.tile([P, CHUNK], f32, tag="mm512", name="mmps")
                        for _ in range(NCH)
                    ]
                for k in range(CT2):
                    lhsT = w1_s[:, k, m * P : (m + 1) * P]
                    for j in range(NCH):
                        nc.tensor.matmul(
                            pss[j],
                            lhsT,
                            w1_rhs[k][:, j * CHUNK : (j + 1) * CHUNK],
                            start=(k == 0),
                            stop=(k == CT2 - 1),
                        )
                for j in range(NCH):
                    sl = slice(j * CHUNK, (j + 1) * CHUNK)
                    nc.vector.tensor_scalar_add(
                        h1[p][:, m, sl], pss[j], b1_s[:, m : m + 1]
                    )

            def emit_sigx(p):
                with nc.allow_low_precision(reason="bf16 colsums feed bf16 GEMM"):
                    for c in range(CT):
                        nc.vector.reduce_sum(
                            sigx[:, c, p : p + 1],
                            x_s[p][:, c, :],
                            axis=mybir.AxisListType.X,
                        )

            # ---- software-pipelined schedule: pair 1's exp-bound attention
            # window absorbs pair 0's W1 GEMM.
            emit_kconv(0)
            emit_attention(0, inline_denoms=True)
            emit_recips(0)
            emit_kconv(1)
            emit_uphase(0)
            emit_sigx(0)
            emit_attention(1, inline_denoms=False,
                           fill_cb=lambda m: emit_w1_block(0, m))
            emit_recips(1)
            emit_squares(0)
            emit_uphase(1)
            emit_sigx(1)
            for m in range(CT2):
                emit_w1_block(1, m, use_quads=True)
                nc.scalar.activation(
                    sq_scr,
                    h1[1][:, m, :],
                    AF.Square,
                    accum_out=ssq[:, m, 1:2],
                )
            # Preload the Ln ACT table while ScalarE idles so the BN-stats Ln
            # after the AllReduce pays no table switch.
            nc.scalar.activation(warm, warm, AF.Ln)

            # ---- BN statistics: sum_n h1 = W1eff @ colsum([x; u]) + N*b1 ----
            sig_t = persist.tile([P, CT2], bf16, tag="sig_t")
            nc.vector.tensor_add(sig_t[:, 0:CT], sigx[:, :, 0], sigx[:, :, 1])
            with nc.allow_low_precision(reason="bf16 colsums feed bf16 GEMM"):
                for c in range(CT):
                    nc.vector.reduce_sum(
                        sig_t[:, CT + c : CT + c + 1],
                        sigu[:, c, :],
                        axis=mybir.AxisListType.X,
                    )
            pstat = psum.tile([P, CHUNK], f32, tag="mm512", name="pstat")
            for m in range(CT2):
                for k in range(CT2):
                    nc.tensor.matmul(
                        pstat[:, m : m + 1],
                        w1_s[:, k, m * P : (m + 1) * P],
                        sig_t[:, k : k + 1],
                        start=(k == 0),
                        stop=(k == CT2 - 1),
                    )
            stats_l = persist.tile([P, 2 * CT2], f32, tag="stats_l")
            nb1 = persist.tile([P, CT2], f32, tag="nb1")
            nc.vector.tensor_scalar_mul(nb1, b1_s, float(2 * N))
            rg = persist.tile([P, CT2], f32, tag="rg")
            with nc.allow_low_precision(reason="gamma reciprocal, f32"):
                nc.vector.reciprocal(rg, gm_s)
            nc.vector.tensor_add(stats_l[:, 0:CT2], pstat[:, 0:CT2], nb1)
            for m in range(CT2):
                nc.vector.tensor_add(
                    stats_l[:, CT2 + m : CT2 + m + 1],
                    ssq[:, m, 0:1],
                    ssq[:, m, 1:2],
                )
            # Cross-core exchange of the 4 KB BN stats: AllGather (single
            # ncfw phase, no reduce step) + a local 8-way tree sum.
            nc.sync.dma_start(out=cc_in[:], in_=stats_l[:])
            nc.gpsimd.collective_compute(
                "AllGather",
                ALU.bypass,
                replica_groups=[list(range(NCORES))],
                ins=[cc_in[:].opt()],
                outs=[cc_out[:].opt()],
            )
            stats_a = persist.tile([P, NCORES, 2 * CT2], f32, tag="stats_a")
            for c in range(NCORES):
                eng = nc.sync if c % 2 == 0 else nc.gpsimd
                eng.dma_start(out=stats_a[:, c, :], in_=cc_out[c])
            for step in (4, 2, 1):
                nc.vector.tensor_add(
                    stats_a[:, 0:step, :],
                    stats_a[:, 0:step, :],
                    stats_a[:, step : 2 * step, :],
                )
            stats_g = stats_a[:, 0, :]

            count = float(B * H * N)
            mom = persist.tile([P, 2 * CT2], f32, tag="mom")
            nc.vector.tensor_scalar_mul(mom, stats_g, 1.0 / count)
            var = persist.tile([P, CT2], f32, tag="var")
            nc.vector.tensor_mul(var, mom[:, 0:CT2], mom[:, 0:CT2])
            nc.vector.tensor_sub(var, mom[:, CT2 : 2 * CT2], var)
            nc.vector.tensor_scalar_add(var, var, EPS)
            # rsqrt = exp(-0.5 ln(var+eps)): Ln table preloaded above.
            lnv = persist.tile([P, CT2], f32, tag="lnv")
            nc.scalar.activation(lnv, var, AF.Ln)
            inv = persist.tile([P, CT2], f32, tag="inv")
            nc.scalar.activation(inv, lnv, AF.Exp, scale=-0.5)
            # sigma = (var+eps)*rsqrt(var+eps): no second Exp needed.
            sg = persist.tile([P, CT2], f32, tag="sg")
            nc.vector.tensor_mul(sg, var, inv)
            # BN folded into W2 (gamma > 0): w2f = w2 * (gamma/sigma) per
            # input channel; relu threshold thr = beta*sigma/gamma - mu.
            scl = persist.tile([P, CT2], f32, tag="scl")
            nc.vector.tensor_mul(scl, gm_s, inv)
            thr = persist.tile([P, CT2], f32, tag="thr")
            nc.vector.tensor_mul(thr, bt_s, sg)
            nc.vector.tensor_mul(thr, thr, rg)
            nc.vector.tensor_sub(thr, thr, mom[:, 0:CT2])
            w2f = persist.tile([P, CT2, D], bf16, tag="w2f")
            for k in range(CT2):
                nc.vector.tensor_scalar_mul(
                    w2f[:, k, :], w2_s[:, k, :], scl[:, k : k + 1]
                )

            # ---- pass 2: relu(h1 + thr) per n-chunk (ScalarE/DVE split),
            # double-buffered chunk tiles so the pairs overlap, then the
            # rescaled W2 GEMM with PSUM slots from both pools.
            for p in range(PAIRS_PER_CORE):
                o_big = work.tile(
                    [P, CT, N], bf16, tag="o_big", name=f"o_big{p}"
                )
                for j in range(NCH):
                    sl = slice(j * CHUNK, (j + 1) * CHUNK)
                    h1n = work.tile(
                        [P, CT2, CHUNK], bf16, tag="h1n", name=f"h1n{p}{j}"
                    )
                    for m in range(CT2):
                        if m < 2:
                            nc.scalar.activation(
                                h1n[:, m, :],
                                h1[p][:, m, sl],
                                AF.Relu,
                                bias=thr[:, m : m + 1],
                            )
                        else:
                            nc.vector.tensor_scalar(
                                h1n[:, m, :],
                                h1[p][:, m, sl],
                                thr[:, m : m + 1],
                                0.0,
                                op0=ALU.add,
                                op1=ALU.max,
                            )
                    for c in range(CT):
                        if c == 0:
                            qt = quad.tile([P, 2, CHUNK], f32, tag="q2", name="q2")
                            ps = qt[:, 0, :]
                        else:
                            ps = psum.tile([P, CHUNK], f32, tag="mm512", name="mmps")
                        for k in range(CT2):
                            nc.tensor.matmul(
                                ps,
                                w2f[:, k, c * P : (c + 1) * P],
                                h1n[:, k, :],
                                start=(k == 0),
                                stop=(k == CT2 - 1),
                            )
                        nc.vector.tensor_scalar_add(
                            o_big[:, c, sl], ps, b2_s[:, c : c + 1]
                        )
                        nc.sync.dma_start(out=out[p, c, :, sl], in_=o_big[:, c, sl])

    nc.finalize()
    return nc
def _get_nc():
    if "nc" not in _CACHE:
        _CACHE["nc"] = build_bass()
    return _CACHE["nc"]


def _prep_inputs(inputs):
    """Host-side shard/fold/transpose/cast. Returns in_maps for the 8 cores."""
    x = np.asarray(inputs["x"], np.float32)
    source = np.asarray(inputs["source"], np.float32)

    # [B, D, H, N] -> [B*H pairs, P, CT, N] (partition-major)
    def to_pairs(a):
        a = a.transpose(0, 2, 1, 3).reshape(B * H, CT, P, N)
        return np.ascontiguousarray(a.transpose(0, 2, 1, 3))

    xp_f = to_pairs(x)
    sp_f = to_pairs(source)
    xp = xp_f.astype(BF16)

    # [pairs, P, CT, N] -> [pairs, P, NCH, CT, CHUNK] (DoubleRow-contiguous)
    def to_chunks(a):
        return np.ascontiguousarray(
            a.reshape(B * H, P, CT, NCH, CHUNK).transpose(0, 1, 3, 2, 4)
        ).astype(FP8)

    xp8 = to_chunks(xp_f)
    sp8 = to_chunks(sp_f)
    # s^T: [pairs, P(m%128), MT, D]
    sT = source.transpose(0, 2, 3, 1).reshape(B * H, MT, P, D)
    sT8 = np.ascontiguousarray(sT.transpose(0, 2, 1, 3)).astype(FP8)

    def lhsT(w, dt, scale=1.0):
        wT = np.ascontiguousarray(np.asarray(w, np.float32).T * scale)
        cin, cout = wT.shape
        a = wT.reshape(cin // P, P, cout).transpose(1, 0, 2)
        return np.ascontiguousarray(a).astype(dt)

    def vcol(b):
        return np.asarray(b, np.float32).reshape(-1, P).T  # [P, kt]

    Wq = np.asarray(inputs["Wq"], np.float32)
    Wk = np.asarray(inputs["Wk"], np.float32)
    Wv = np.asarray(inputs["Wv"], np.float32)
    Wm = np.asarray(inputs["Wm"], np.float32)
    W1 = np.asarray(inputs["W1"], np.float32)
    G = Wq.T @ Wk
    WU = W1[:, D:] @ Wm @ Wv
    W1eff = np.concatenate([W1[:, :D], WU / WS], axis=1)
    bm_eff = Wm @ np.asarray(inputs["bv"], np.float32) + np.asarray(
        inputs["bm"], np.float32
    )
    b1_eff = np.asarray(inputs["b1"], np.float32) + W1[:, D:] @ bm_eff
    bkp = WS * (Wq.T @ np.asarray(inputs["bk"], np.float32))

    vecs = np.zeros((P, 24), np.float32)
    vecs[:, 0:2] = vcol(bkp)
    vecs[:, 8:12] = vcol(b1_eff)
    vecs[:, 12:14] = vcol(inputs["b2"])
    vecs[:, 14:18] = vcol(inputs["gamma"])
    vecs[:, 18:22] = vcol(inputs["beta"])

    common = {
        "gT": lhsT(G, FP8, WS),
        "w1T": lhsT(W1eff, BF16),
        "w2T": lhsT(inputs["W2"], BF16),
        "vecs": vecs,
    }
    in_maps = []
    for i in range(NCORES):
        m = dict(common)
        pp = slice(i * PAIRS_PER_CORE, (i + 1) * PAIRS_PER_CORE)
        m["xb"] = np.ascontiguousarray(xp[pp])
        m["x8"] = np.ascontiguousarray(xp8[pp])
        m["s8"] = np.ascontiguousarray(sp8[pp])
        m["sT8"] = np.ascontiguousarray(sT8[pp])
        in_maps.append(m)
    return in_maps


def run_on_hw(inputs, trace=False, **kw):
    nc = _get_nc()
    in_maps = _prep_inputs(inputs)
    res = run_bass_kernel_spmd(
        nc, in_maps, core_ids=list(range(NCORES)), trace=trace, **kw
    )
    outs = res.results
    full = np.empty((B, H, D, N), np.float32)
    for i in range(NCORES):
        o = np.asarray(outs[i]["out"]).astype(np.float32).reshape(PAIRS_PER_CORE, D, N)
        for jp in range(PAIRS_PER_CORE):
            gp = i * PAIRS_PER_CORE + jp
            full[gp // H, gp % H] = o[jp]
    return full.transpose(0, 2, 1, 3), res


def kernel(**inputs) -> np.ndarray:
    out, _ = run_on_hw(inputs, trace=False)
    return out
